# revision 21
# baseline (speedup 1.0000x reference)
"""Causal single-head attention on 8 Trainium2 NeuronCores.

Problem: x [32, 2048, 384] f32, Wq/Wk/Wv [384, 64] f32.
  q/k/v = x @ W;  out = softmax_causal(q k^T / sqrt(64)) @ v   -> [32, 2048, 64]

Strategy: data-parallel over batch (4 batches per core), weights replicated,
bf16 matmul path (fp32 accumulation), no collectives.

Per batch on one core (T=2048, C=384, H=64):
  - host pre-transposes x to xT [C, T] and casts to bf16
  - projection pass 1 with stationary [Wq|Wk] -> psum rows 0:64 = qT,
    rows 64:128 = kT ("hi" copy); pass 2 with [Wv|Wq] -> vT lo + qT hi.
    One extra SBUF->SBUF DMA makes the base-0 kT copy. This gives every
    operand at both partition bases so the causal score matmuls can run as
    row-tiled PAIRS (two K=64 matmuls concurrently in the 128-deep array).
  - score strips are produced in DIAGONAL order (strip (p, s) covers
    t-chunk p//2 + s; diagonal d = all strips covering t-chunk d), exp'd
    per strip (scale=1/8 fused), with matmul N trimmed to the causal
    region at 128 granularity (no memsets needed; garbage cols are never
    read by AV)
  - AV is t-chunk-major: for t-chunk j accumulate s-chunks 0..4j+3 into
    one PSUM bank [65, 512] (v-with-ones-column stationary, row 64 =
    softmax denominator); per-s-chunk start offset at 128 granularity
  - emission interleaves: diag d strips, AV j=d-1, and next batch's
    projections are woven so TensorE always has work while ScalarE
    (the exp pacer) drains strips; PSUM: 2x strip (2 banks each),
    2x proj (1 bank), 2x AV-accum/transpose (1 bank) = 8 banks
  - PE-transpose outT -> [t, 65], reciprocal of col 64, broadcast multiply,
    one DMA per 512-row block, f32 out
  - optional: a subset of strips (GP_STRIPS) does exp on GpSimd via the
    Schraudolph bit trick (i16 = round(A*score + B) viewed as bf16),
    offloading the ScalarE bottleneck
"""

import sys

sys.path.insert(0, "/opt/trn_rl_repo")

import numpy as np
import ml_dtypes

import concourse.bass as bass
import concourse.mybir as mybir
import concourse.tile as tile
from concourse import bacc
from concourse.bass_utils import run_bass_kernel_spmd

BF16 = mybir.dt.bfloat16
F32 = mybir.dt.float32
I16 = mybir.dt.int16
NP_BF16 = ml_dtypes.bfloat16

B, T_FULL, C, H = 32, 2048, 384, 64
N_CORES = 8
B_LOC = B // N_CORES
SCALE = float(H) ** -0.5
Exp = mybir.ActivationFunctionType.Exp

# Schraudolph exp-to-bf16 constants: bf16_bits(exp(s*SCALE)) ~
# round(A_S * s + B_S) as int16.  C_S tunes the mean relative error.
C_S = 8.5
A_S = 128.0 / np.log(2.0) * SCALE
B_S = 127.0 * 128.0 - C_S

# strips (p, sidx) whose exp runs on DVE (Schraudolph) instead of ScalarE
GP_STRIPS = {(0, 1), (0, 2), (0, 3), (1, 1)}


def build_nc(b_loc=B_LOC, t=T_FULL):
    """Build the per-core Bass program (SPMD: same program on all cores)."""
    assert t % 512 == 0
    nc = bacc.Bacc(None, target_bir_lowering=False)
    cc = C // 128          # contraction chunks for projections
    ns = t // 128          # number of 128-wide s-chunks
    nt = t // 512          # number of 512-wide t-chunks
    npair = ns // 2        # s-chunk pairs (even chunk -> base 0, odd -> base 64)

    xT = nc.declare_dram_parameter("xT", [b_loc, C, t], BF16, isOutput=False)
    # host-packed projection stationaries [Wq|Wk], [Wv|Wq] per c-chunk
    wqk_d = nc.declare_dram_parameter("Wqk", [128, C // 128, 128], BF16,
                                      isOutput=False)
    wvq_d = nc.declare_dram_parameter("Wvq", [128, C // 128, 128], BF16,
                                      isOutput=False)
    id16_d = nc.declare_dram_parameter("ident16", [128, 128], BF16, isOutput=False)
    id32_d = nc.declare_dram_parameter("ident32", [128, 128], F32, isOutput=False)
    mask_d = nc.declare_dram_parameter("mask", [128, 128], BF16, isOutput=False)
    outp = nc.declare_dram_parameter("out", [b_loc, t, H], F32, isOutput=True)

    with tile.TileContext(nc) as tc:
        with (
            tc.tile_pool(name="consts", bufs=1) as consts,
            tc.tile_pool(name="xt", bufs=6) as p_xt,
            tc.tile_pool(name="qk", bufs=2) as p_qk,
            tc.tile_pool(name="vv", bufs=2) as p_v,
            tc.tile_pool(name="exp", bufs=3) as p_exp,
            tc.tile_pool(name="oo", bufs=2) as p_o,
            tc.tile_pool(name="ps_strip", bufs=2, space="PSUM") as ps_strip,
            tc.tile_pool(name="ps_proj", bufs=2, space="PSUM") as ps_proj,
            tc.tile_pool(name="ps_av", bufs=2, space="PSUM") as ps_av,
        ):
            # per-batch live state
            state = {}

            def emit_xt_dma(b, by_tj=False):
                xts = [
                    p_xt.tile([128, t], BF16, tag="xt", name=f"xt{b}{c}")
                    for c in range(cc)
                ]
                if by_tj:
                    # t-chunk-major loads so proj tj=0 can start asap
                    for tj in range(nt):
                        tr = slice(512 * tj, 512 * (tj + 1))
                        for c in range(cc):
                            nc.sync.dma_start(
                                out=xts[c][:, tr],
                                in_=xT[b, 128 * c:128 * (c + 1), tr],
                            )
                else:
                    for c in range(cc):
                        nc.sync.dma_start(
                            out=xts[c], in_=xT[b, 128 * c:128 * (c + 1), :]
                        )
                return xts

            # ---- startup: weights, then batch-0 xT t-chunk-major ----
            wqk = consts.tile([128, cc, 128], BF16)
            nc.sync.dma_start(out=wqk, in_=wqk_d[:, :, :])
            wvq = consts.tile([128, cc, 128], BF16)
            nc.sync.dma_start(out=wvq, in_=wvq_d[:, :, :])
            xts = emit_xt_dma(0, by_tj=True)
            dmask = consts.tile([128, 128], BF16)
            nc.sync.dma_start(out=dmask, in_=mask_d[:, :])
            ident16 = consts.tile([128, 128], BF16)
            nc.sync.dma_start(out=ident16, in_=id16_d[:, :])
            ident32 = consts.tile([128, 128], F32)
            nc.sync.dma_start(out=ident32, in_=id32_d[:, :])

            def emit_strip(b, p, sidx):
                """Score matmul pair + exp for strip sidx of pair p.
                Strip covers t-chunk p//2 + sidx."""
                st = state[b]
                g1, g2, klo = st["g1"], st["g2"], st["klo"]
                jd = p // 2
                t0 = 512 * jd
                i0, i1 = 2 * p, 2 * p + 1
                w = t - t0            # expT tile width for this pair
                if sidx == 0:
                    expT = p_exp.tile([128, 2, w], BF16, tag=f"exp{jd}",
                                      name=f"exp{b}_{p}")
                    st["expT"][p] = expT
                else:
                    expT = st["expT"][p]
                ts0 = t0 + 512 * sidx
                ps_s = ps_strip.tile([128, 2, 512], F32, tag="strip",
                                     name=f"pss{b}{p}{sidx}")
                if sidx == 0:
                    # causal trim: chunk i valid from col 128*i (abs t)
                    off0 = 128 * i0 - ts0        # 0 (even p) / 256 (odd p)
                    off1 = 128 * i1 - ts0        # 128 (even p) / 384 (odd p)
                else:
                    off0 = off1 = 0
                nc.tensor.matmul(
                    ps_s[:, 0, off0:512],
                    klo[:, 128 * i0:128 * (i0 + 1)],
                    g1[0:64, ts0 + off0:ts0 + 512],
                    start=True, stop=True,
                )
                nc.tensor.matmul(
                    ps_s[:, 1, off1:512],
                    g1[64:128, 128 * i1:128 * (i1 + 1)],
                    g2[64:128, ts0 + off1:ts0 + 512],
                    start=True, stop=True,
                )
                # exp over the union of valid cols (parity-1 cols in
                # [off0, off1) are garbage but never read by AV)
                eoff = off0
                dst = expT[:, :, ts0 - t0 + eoff:ts0 - t0 + 512]
                src = ps_s[:, :, eoff:512]
                if (p, sidx) in GP_STRIPS:
                    # Schraudolph bit-trick exp on DVE (GpSimd can't read PSUM)
                    nc.vector.tensor_scalar(
                        dst.bitcast(I16), src, A_S, B_S,
                        op0=mybir.AluOpType.mult, op1=mybir.AluOpType.add,
                    )
                else:
                    nc.scalar.activation(dst, src, Exp, scale=SCALE)
                if sidx == 0:
                    # mask the two diagonal blocks
                    d0 = 128 * i0 - t0
                    nc.vector.tensor_mul(
                        expT[:, 0, d0:d0 + 128], expT[:, 0, d0:d0 + 128], dmask
                    )
                    d1 = 128 * i1 - t0
                    nc.vector.tensor_mul(
                        expT[:, 1, d1:d1 + 128], expT[:, 1, d1:d1 + 128], dmask
                    )

            def emit_p3(b, j, outT_ps):
                """Transpose outT[65, 512] -> [t,65], normalize, DMA out."""
                outTn = p_o.tile([65, 512], F32, tag="outTn", name=f"otn{b}{j}")
                nc.vector.tensor_copy(outTn, outT_ps)
                ps_o = ps_av.tile([128, 4, 65], F32, tag="av", name=f"pso{b}{j}")
                for tt in range(4):
                    nc.tensor.transpose(
                        ps_o[:, tt, :],
                        outTn[:, 128 * tt:128 * (tt + 1)],
                        ident32[0:65, 0:65],
                    )
                zrec = p_o.tile([128, 4], F32, tag="zrec", bufs=4, name=f"zr{b}{j}")
                nc.vector.reciprocal(zrec, ps_o[:, :, H:H + 1])
                o_sb = p_o.tile([128, 4, H], F32, tag="o_sb", bufs=4,
                                name=f"os{b}{j}")
                zbc = bass.AP(
                    tensor=zrec.tensor, offset=zrec.offset,
                    ap=[zrec.ap[0], zrec.ap[1], [0, H]],
                )
                nc.vector.tensor_tensor(
                    out=o_sb, in0=ps_o[:, :, 0:H], in1=zbc,
                    op=mybir.AluOpType.mult,
                )
                dst = outp[b, 512 * j:512 * (j + 1), :].rearrange(
                    "(tt tl) h -> tl tt h", tl=128
                )
                nc.sync.dma_start(out=dst, in_=o_sb)

            def emit_av_pair(b, j, p, outT):
                """AV matmuls for s-chunks 2p, 2p+1 into t-chunk j's
                accumulator."""
                st = state[b]
                vaug = st["vaug"]
                t0 = 512 * (p // 2)
                expT = st["expT"][p]
                n_i = 4 * j + 4
                for i in (2 * p, 2 * p + 1):
                    # causal trim: chunk i contributes from col 128*i
                    off = max(0, 128 * i - 512 * j)
                    c0 = 512 * j + off - t0
                    nc.tensor.matmul(
                        outT[:, off:512],
                        vaug[:, i, :],
                        expT[:, i % 2, c0:c0 + 512 - off],
                        start=(i == 0), stop=(i == n_i - 1),
                    )

            # ---------------- schedule ----------------
            # Diagonal d of batch b = strips (p, d - p//2) for p = 0..2d+1;
            # AV for t-chunk d consumes exactly those strips (pair p's AV
            # matmuls read pair p's diag-d strip).  Weave: strip p, then AV
            # of strip p-2 (exp'd two slots ago - ScalarE latency hidden),
            # plus filler PE work (next batch's projections) spread evenly.
            def emit_diag(b, d, fillers, strips_left):
                st = state[b]
                outT = ps_av.tile([65, 512], F32, tag="av", name=f"outT{b}{d}")
                st["outT"][d] = outT
                pend = []
                n_strips = 2 * d + 2
                for p in range(n_strips):
                    emit_strip(b, p, d - p // 2)
                    pend.append(p)
                    # spread remaining fillers evenly over remaining strips
                    want = -(-len(fillers) // strips_left[0]) if fillers else 0
                    for _ in range(want):
                        fillers.pop(0)()
                    strips_left[0] -= 1
                    if len(pend) > 3:
                        emit_av_pair(b, d, pend.pop(0), outT)
                while pend:
                    emit_av_pair(b, d, pend.pop(0), outT)
                emit_p3(b, d, outT)

            def proj_fillers(b, xts):
                """Return a list of zero-arg callables that together emit
                proj(b) + vaug(b); safe to run spread across the previous
                batch's diagonals.  Ordered so the first strips' operands
                (g1/klo/g2 of t-chunk 0) come first."""
                g1 = p_qk.tile([128, t], BF16, tag="g1", name=f"g1_{b}")
                g2 = p_qk.tile([128, t], BF16, tag="g2", name=f"g2_{b}")
                klo = p_qk.tile([64, t], BF16, tag="klo", name=f"klo{b}")
                vaug = p_v.tile([128, ns, 65], BF16, tag="vaug", name=f"vaug{b}")
                state[b] = dict(g1=g1, g2=g2, klo=klo, vaug=vaug,
                                expT={}, outT={})
                units = []

                def proj_unit(tj, w_sb, g_sb, is_g1):
                    def f():
                        tr = slice(512 * tj, 512 * (tj + 1))
                        ps_g = ps_proj.tile([128, 512], F32, tag="proj",
                                            name=f"psg{b}{tj}{is_g1}")
                        for c in range(cc):
                            nc.tensor.matmul(
                                ps_g, w_sb[:, c, :], xts[c][:, tr],
                                start=(c == 0), stop=(c == cc - 1),
                            )
                        nc.vector.tensor_copy(g_sb[:, tr], ps_g)
                        if is_g1:
                            # per-t-chunk base-0 copy of kT
                            nc.sync.dma_start(out=klo[:, tr], in_=g1[64:128, tr])
                    return f

                def vaug_unit(st0):
                    def f():
                        if st0 == 0:
                            nc.gpsimd.memset(vaug[:, :, H:H + 1], 1.0)
                        # 4 transposes into one PSUM bank, one batched copy
                        ps_tr = ps_proj.tile([128, 4, 64], BF16, tag="proj",
                                             name=f"pstr{b}{st0}")
                        for q in range(4):
                            st_ = st0 + q
                            nc.tensor.transpose(
                                ps_tr[:, q, :], g2[0:64, 128 * st_:128 * (st_ + 1)],
                                ident16[0:H, 0:H]
                            )
                        nc.vector.tensor_copy(vaug[:, st0:st0 + 4, 0:H], ps_tr)
                    return f

                for tj in range(nt):
                    units.append(proj_unit(tj, wqk, g1, True))
                    units.append(proj_unit(tj, wvq, g2, False))
                    units.append(vaug_unit(4 * tj))
                return units

            # b=0: emit projections up front; vaug units become diag fillers
            units0 = proj_fillers(0, xts)
            fillers = []
            for i, u in enumerate(units0):
                if i % 3 < 2:
                    u()
                else:
                    fillers.append(u)
            n_strips_total = sum(2 * d + 2 for d in range(nt))
            for b in range(b_loc):
                if b + 1 < b_loc:
                    xts = emit_xt_dma(b + 1)
                    fillers = fillers + proj_fillers(b + 1, xts)
                strips_left = [n_strips_total]
                for d in range(nt):
                    emit_diag(b, d, fillers, strips_left)
                for u in fillers:
                    u()
                fillers = []
                del state[b]

    nc.compile()
    return nc


def _shard_inputs(x, Wk, Wq, Wv, b_loc=B_LOC, t=T_FULL):
    ident32 = np.eye(128, dtype=np.float32)
    ident16 = ident32.astype(NP_BF16)
    mask = np.triu(np.ones((128, 128), dtype=np.float32)).astype(NP_BF16)
    wq16 = np.asarray(Wq, dtype=np.float32)
    wk16 = np.asarray(Wk, dtype=np.float32)
    wv16 = np.asarray(Wv, dtype=np.float32)
    cc = C // 128
    # packed stationaries [128, cc, 128]: [Wq|Wk] and [Wv|Wq] per c-chunk
    wqk = np.concatenate(
        [wq16.reshape(cc, 128, H), wk16.reshape(cc, 128, H)], axis=2
    ).transpose(1, 0, 2)
    wvq = np.concatenate(
        [wv16.reshape(cc, 128, H), wq16.reshape(cc, 128, H)], axis=2
    ).transpose(1, 0, 2)
    wqk = np.ascontiguousarray(wqk).astype(NP_BF16)
    wvq = np.ascontiguousarray(wvq).astype(NP_BF16)
    n_cores = x.shape[0] // b_loc
    xs = np.asarray(x, dtype=np.float32).reshape(n_cores, b_loc, t, C)
    in_maps = []
    for m in range(n_cores):
        xT = np.ascontiguousarray(xs[m].transpose(0, 2, 1)).astype(NP_BF16)
        in_maps.append({
            "xT": xT, "Wqk": wqk, "Wvq": wvq,
            "ident16": ident16, "ident32": ident32, "mask": mask,
        })
    return in_maps


def _run(x, Wk, Wq, Wv, trace=False, **spmd_kwargs):
    nc = build_nc()
    in_maps = _shard_inputs(x, Wk, Wq, Wv)
    res = run_bass_kernel_spmd(
        nc, in_maps, core_ids=list(range(N_CORES)), trace=trace, **spmd_kwargs
    )
    out = np.concatenate([res.results[m]["out"] for m in range(N_CORES)], axis=0)
    return np.ascontiguousarray(out, dtype=np.float32), res


def kernel(x, Wk, Wq, Wv):
    out, _ = _run(x, Wk, Wq, Wv)
    return out


# revision 25
# speedup vs baseline: 1.0380x; 1.0380x over previous
"""Causal single-head attention on 8 Trainium2 NeuronCores.

Problem: x [32, 2048, 384] f32, Wq/Wk/Wv [384, 64] f32.
  q/k/v = x @ W;  out = softmax_causal(q k^T / sqrt(64)) @ v   -> [32, 2048, 64]

Strategy: data-parallel over batch (4 batches per core), weights replicated,
bf16 matmul path (fp32 accumulation), no collectives.

Per batch on one core (T=2048, C=384, H=64):
  - host pre-transposes x to xT [C, T] and casts to bf16
  - projection pass 1 with stationary [Wq|Wk] -> psum rows 0:64 = qT,
    rows 64:128 = kT ("hi" copy); pass 2 with [Wv|Wq] -> vT lo + qT hi.
    One extra SBUF->SBUF DMA makes the base-0 kT copy. This gives every
    operand at both partition bases so the causal score matmuls can run as
    row-tiled PAIRS (two K=64 matmuls concurrently in the 128-deep array).
  - score strips are produced in DIAGONAL order (strip (p, s) covers
    t-chunk p//2 + s; diagonal d = all strips covering t-chunk d), exp'd
    per strip (scale=1/8 fused), with matmul N trimmed to the causal
    region at 128 granularity (no memsets needed; garbage cols are never
    read by AV)
  - AV is t-chunk-major: for t-chunk j accumulate s-chunks 0..4j+3 into
    one PSUM bank [65, 512] (v-with-ones-column stationary, row 64 =
    softmax denominator); per-s-chunk start offset at 128 granularity
  - emission interleaves: diag d strips, AV j=d-1, and next batch's
    projections are woven so TensorE always has work while ScalarE
    (the exp pacer) drains strips; PSUM: 2x strip (2 banks each),
    2x proj (1 bank), 2x AV-accum/transpose (1 bank) = 8 banks
  - PE-transpose outT -> [t, 65], reciprocal of col 64, broadcast multiply,
    one DMA per 512-row block, f32 out
  - optional: a subset of strips (GP_STRIPS) does exp on GpSimd via the
    Schraudolph bit trick (i16 = round(A*score + B) viewed as bf16),
    offloading the ScalarE bottleneck
"""

import sys

sys.path.insert(0, "/opt/trn_rl_repo")

import numpy as np
import ml_dtypes

import concourse.bass as bass
import concourse.mybir as mybir
import concourse.tile as tile
from concourse import bacc
from concourse.bass_utils import run_bass_kernel_spmd

BF16 = mybir.dt.bfloat16
F32 = mybir.dt.float32
I16 = mybir.dt.int16
NP_BF16 = ml_dtypes.bfloat16

B, T_FULL, C, H = 32, 2048, 384, 64
N_CORES = 8
B_LOC = B // N_CORES
SCALE = float(H) ** -0.5
Exp = mybir.ActivationFunctionType.Exp

# Schraudolph exp-to-bf16 constants: bf16_bits(exp(s*SCALE)) ~
# round(A_S * s + B_S) as int16.  C_S tunes the mean relative error.
C_S = 8.5
A_S = 128.0 / np.log(2.0) * SCALE
B_S = 127.0 * 128.0 - C_S

# strips (p, sidx) whose exp runs on DVE (Schraudolph) instead of ScalarE
GP_STRIPS = {(0, 1), (0, 2)}


def build_nc(b_loc=B_LOC, t=T_FULL):
    """Build the per-core Bass program (SPMD: same program on all cores)."""
    assert t % 512 == 0
    nc = bacc.Bacc(None, target_bir_lowering=False)
    cc = C // 128          # contraction chunks for projections
    ns = t // 128          # number of 128-wide s-chunks
    nt = t // 512          # number of 512-wide t-chunks
    npair = ns // 2        # s-chunk pairs (even chunk -> base 0, odd -> base 64)

    xT = nc.declare_dram_parameter("xT", [b_loc, C, t], BF16, isOutput=False)
    # host-packed projection stationaries [Wq|Wk], [Wv|Wq] per c-chunk
    wqk_d = nc.declare_dram_parameter("Wqk", [128, C // 128, 128], BF16,
                                      isOutput=False)
    wvq_d = nc.declare_dram_parameter("Wvq", [128, C // 128, 128], BF16,
                                      isOutput=False)
    id16_d = nc.declare_dram_parameter("ident16", [128, 128], BF16, isOutput=False)
    id32_d = nc.declare_dram_parameter("ident32", [128, 128], F32, isOutput=False)
    mask_d = nc.declare_dram_parameter("mask", [128, 128], BF16, isOutput=False)
    outp = nc.declare_dram_parameter("out", [b_loc, t, H], F32, isOutput=True)

    with tile.TileContext(nc) as tc:
        with (
            tc.tile_pool(name="consts", bufs=1) as consts,
            tc.tile_pool(name="xt", bufs=6) as p_xt,
            tc.tile_pool(name="qk", bufs=2) as p_qk,
            tc.tile_pool(name="vv", bufs=2) as p_v,
            tc.tile_pool(name="exp", bufs=3) as p_exp,
            tc.tile_pool(name="oo", bufs=2) as p_o,
            tc.tile_pool(name="ps_strip", bufs=2, space="PSUM") as ps_strip,
            tc.tile_pool(name="ps_proj", bufs=2, space="PSUM") as ps_proj,
            tc.tile_pool(name="ps_av", bufs=2, space="PSUM") as ps_av,
        ):
            # per-batch live state
            state = {}

            def emit_xt_dma(b, by_tj=False):
                xts = [
                    p_xt.tile([128, t], BF16, tag="xt", name=f"xt{b}{c}")
                    for c in range(cc)
                ]
                if by_tj:
                    # t-chunk-major loads so proj tj=0 can start asap
                    for tj in range(nt):
                        tr = slice(512 * tj, 512 * (tj + 1))
                        for c in range(cc):
                            nc.sync.dma_start(
                                out=xts[c][:, tr],
                                in_=xT[b, 128 * c:128 * (c + 1), tr],
                            )
                else:
                    for c in range(cc):
                        nc.sync.dma_start(
                            out=xts[c], in_=xT[b, 128 * c:128 * (c + 1), :]
                        )
                return xts

            # ---- startup: weights, then batch-0 xT t-chunk-major ----
            wqk = consts.tile([128, cc, 128], BF16)
            nc.sync.dma_start(out=wqk, in_=wqk_d[:, :, :])
            wvq = consts.tile([128, cc, 128], BF16)
            nc.sync.dma_start(out=wvq, in_=wvq_d[:, :, :])
            xts = emit_xt_dma(0, by_tj=True)
            dmask = consts.tile([128, 128], BF16)
            nc.sync.dma_start(out=dmask, in_=mask_d[:, :])
            ident16 = consts.tile([128, 128], BF16)
            nc.sync.dma_start(out=ident16, in_=id16_d[:, :])
            ident32 = consts.tile([128, 128], F32)
            nc.sync.dma_start(out=ident32, in_=id32_d[:, :])

            def emit_strip(b, p, sidx):
                """Score matmul pair + exp for strip sidx of pair p.
                Strip covers t-chunk p//2 + sidx."""
                st = state[b]
                g1, g2, klo = st["g1"], st["g2"], st["klo"]
                jd = p // 2
                t0 = 512 * jd
                i0, i1 = 2 * p, 2 * p + 1
                w = t - t0            # expT tile width for this pair
                if sidx == 0:
                    expT = p_exp.tile([128, 2, w], BF16, tag=f"exp{jd}",
                                      name=f"exp{b}_{p}")
                    st["expT"][p] = expT
                else:
                    expT = st["expT"][p]
                ts0 = t0 + 512 * sidx
                ps_s = ps_strip.tile([128, 2, 512], F32, tag="strip",
                                     name=f"pss{b}{p}{sidx}")
                if sidx == 0:
                    # causal trim: chunk i valid from col 128*i (abs t)
                    off0 = 128 * i0 - ts0        # 0 (even p) / 256 (odd p)
                    off1 = 128 * i1 - ts0        # 128 (even p) / 384 (odd p)
                else:
                    off0 = off1 = 0
                nc.tensor.matmul(
                    ps_s[:, 0, off0:512],
                    klo[:, 128 * i0:128 * (i0 + 1)],
                    g1[0:64, ts0 + off0:ts0 + 512],
                    start=True, stop=True,
                )
                nc.tensor.matmul(
                    ps_s[:, 1, off1:512],
                    g1[64:128, 128 * i1:128 * (i1 + 1)],
                    g2[64:128, ts0 + off1:ts0 + 512],
                    start=True, stop=True,
                )
                # exp over the union of valid cols (parity-1 cols in
                # [off0, off1) are garbage but never read by AV)
                eoff = off0
                dst = expT[:, :, ts0 - t0 + eoff:ts0 - t0 + 512]
                src = ps_s[:, :, eoff:512]
                if (p, sidx) in GP_STRIPS:
                    # Schraudolph bit-trick exp on DVE (GpSimd can't read PSUM)
                    nc.vector.tensor_scalar(
                        dst.bitcast(I16), src, A_S, B_S,
                        op0=mybir.AluOpType.mult, op1=mybir.AluOpType.add,
                    )
                else:
                    nc.scalar.activation(dst, src, Exp, scale=SCALE)
                if sidx == 0:
                    # mask both diagonal blocks with one strided op:
                    # dim-1 hop = (parity 1, col d1) - (parity 0, col d0)
                    d0 = 128 * i0 - t0
                    mt = bass.AP(
                        tensor=expT.tensor, offset=expT.offset + d0,
                        ap=[expT.ap[0], [w + 128, 2], [1, 128]],
                    )
                    mk = bass.AP(
                        tensor=dmask.tensor, offset=dmask.offset,
                        ap=[dmask.ap[0], [0, 2], [1, 128]],
                    )
                    nc.vector.tensor_tensor(
                        out=mt, in0=mt, in1=mk, op=mybir.AluOpType.mult
                    )

            def emit_p3(b, j, outT_ps):
                """Transpose outT[65, 512] -> [t,65], normalize, DMA out.
                bf16 transposes (fp32 PE transpose runs 2-pass); the f32
                denominator reciprocal restores precision for the divide."""
                outTn = p_o.tile([65, 512], BF16, tag="outTn", name=f"otn{b}{j}")
                nc.vector.tensor_copy(outTn, outT_ps)
                ps_o = ps_av.tile([128, 4, 66], BF16, tag="av", name=f"pso{b}{j}")
                for tt in range(4):
                    nc.tensor.transpose(
                        ps_o[:, tt, 0:65],
                        outTn[:, 128 * tt:128 * (tt + 1)],
                        ident16[0:65, 0:65],
                    )
                zrec = p_o.tile([128, 4], F32, tag="zrec", bufs=4, name=f"zr{b}{j}")
                nc.vector.reciprocal(zrec, ps_o[:, :, H:H + 1])
                o_sb = p_o.tile([128, 4, H], F32, tag="o_sb", bufs=4,
                                name=f"os{b}{j}")
                zbc = bass.AP(
                    tensor=zrec.tensor, offset=zrec.offset,
                    ap=[zrec.ap[0], zrec.ap[1], [0, H]],
                )
                nc.vector.tensor_tensor(
                    out=o_sb, in0=ps_o[:, :, 0:H], in1=zbc,
                    op=mybir.AluOpType.mult,
                )
                dst = outp[b, 512 * j:512 * (j + 1), :].rearrange(
                    "(tt tl) h -> tl tt h", tl=128
                )
                nc.sync.dma_start(out=dst, in_=o_sb)

            def emit_av_pair(b, j, p, outT):
                """AV matmuls for s-chunks 2p, 2p+1 into t-chunk j's
                accumulator."""
                st = state[b]
                vaug = st["vaug"]
                t0 = 512 * (p // 2)
                expT = st["expT"][p]
                n_i = 4 * j + 4
                for i in (2 * p, 2 * p + 1):
                    # causal trim: chunk i contributes from col 128*i
                    off = max(0, 128 * i - 512 * j)
                    c0 = 512 * j + off - t0
                    nc.tensor.matmul(
                        outT[:, off:512],
                        vaug[:, i, :],
                        expT[:, i % 2, c0:c0 + 512 - off],
                        start=(i == 0), stop=(i == n_i - 1),
                    )

            # ---------------- schedule ----------------
            # Diagonal d of batch b = strips (p, d - p//2) for p = 0..2d+1;
            # AV for t-chunk d consumes exactly those strips (pair p's AV
            # matmuls read pair p's diag-d strip).  Weave: strip p, then AV
            # of strip p-2 (exp'd two slots ago - ScalarE latency hidden),
            # plus filler PE work (next batch's projections) spread evenly.
            def emit_diag(b, d, fillers, strips_left):
                st = state[b]
                outT = ps_av.tile([65, 512], F32, tag="av", name=f"outT{b}{d}")
                st["outT"][d] = outT
                pend = []
                n_strips = 2 * d + 2
                for p in range(n_strips):
                    emit_strip(b, p, d - p // 2)
                    pend.append(p)
                    # spread remaining fillers evenly over remaining strips
                    want = -(-len(fillers) // strips_left[0]) if fillers else 0
                    for _ in range(want):
                        fillers.pop(0)()
                    strips_left[0] -= 1
                    if len(pend) > 3:
                        emit_av_pair(b, d, pend.pop(0), outT)
                while pend:
                    emit_av_pair(b, d, pend.pop(0), outT)
                emit_p3(b, d, outT)

            def proj_fillers(b, xts):
                """Return a list of zero-arg callables that together emit
                proj(b) + vaug(b); safe to run spread across the previous
                batch's diagonals.  Ordered so the first strips' operands
                (g1/klo/g2 of t-chunk 0) come first."""
                g1 = p_qk.tile([128, t], BF16, tag="g1", name=f"g1_{b}")
                g2 = p_qk.tile([128, t], BF16, tag="g2", name=f"g2_{b}")
                klo = p_qk.tile([64, t], BF16, tag="klo", name=f"klo{b}")
                vaug = p_v.tile([128, ns, 65], BF16, tag="vaug", name=f"vaug{b}")
                state[b] = dict(g1=g1, g2=g2, klo=klo, vaug=vaug,
                                expT={}, outT={})
                units = []

                def proj_unit(tj, w_sb, g_sb, is_g1):
                    def f():
                        tr = slice(512 * tj, 512 * (tj + 1))
                        ps_g = ps_proj.tile([128, 512], F32, tag="proj",
                                            name=f"psg{b}{tj}{is_g1}")
                        for c in range(cc):
                            nc.tensor.matmul(
                                ps_g, w_sb[:, c, :], xts[c][:, tr],
                                start=(c == 0), stop=(c == cc - 1),
                            )
                        nc.vector.tensor_copy(g_sb[:, tr], ps_g)
                        if is_g1:
                            # per-t-chunk base-0 copy of kT
                            nc.sync.dma_start(out=klo[:, tr], in_=g1[64:128, tr])
                    return f

                def vaug_unit(st0):
                    def f():
                        if st0 == 0:
                            nc.gpsimd.memset(vaug[:, :, H:H + 1], 1.0)
                        # 4 transposes into one PSUM bank, one batched copy
                        ps_tr = ps_proj.tile([128, 4, 64], BF16, tag="proj",
                                             name=f"pstr{b}{st0}")
                        for q in range(4):
                            st_ = st0 + q
                            nc.tensor.transpose(
                                ps_tr[:, q, :], g2[0:64, 128 * st_:128 * (st_ + 1)],
                                ident16[0:H, 0:H]
                            )
                        nc.vector.tensor_copy(vaug[:, st0:st0 + 4, 0:H], ps_tr)
                    return f

                for tj in range(nt):
                    units.append(proj_unit(tj, wqk, g1, True))
                    units.append(proj_unit(tj, wvq, g2, False))
                    units.append(vaug_unit(4 * tj))
                return units

            # b=0: emit projections up front; vaug units become diag fillers
            units0 = proj_fillers(0, xts)
            fillers = []
            for i, u in enumerate(units0):
                if i % 3 < 2:
                    u()
                else:
                    fillers.append(u)
            n_strips_total = sum(2 * d + 2 for d in range(nt))
            for b in range(b_loc):
                if b + 1 < b_loc:
                    xts = emit_xt_dma(b + 1)
                    fillers = fillers + proj_fillers(b + 1, xts)
                strips_left = [n_strips_total]
                for d in range(nt):
                    emit_diag(b, d, fillers, strips_left)
                for u in fillers:
                    u()
                fillers = []
                del state[b]

    nc.compile()
    return nc


def _shard_inputs(x, Wk, Wq, Wv, b_loc=B_LOC, t=T_FULL):
    ident32 = np.eye(128, dtype=np.float32)
    ident16 = ident32.astype(NP_BF16)
    mask = np.triu(np.ones((128, 128), dtype=np.float32)).astype(NP_BF16)
    wq16 = np.asarray(Wq, dtype=np.float32)
    wk16 = np.asarray(Wk, dtype=np.float32)
    wv16 = np.asarray(Wv, dtype=np.float32)
    cc = C // 128
    # packed stationaries [128, cc, 128]: [Wq|Wk] and [Wv|Wq] per c-chunk
    wqk = np.concatenate(
        [wq16.reshape(cc, 128, H), wk16.reshape(cc, 128, H)], axis=2
    ).transpose(1, 0, 2)
    wvq = np.concatenate(
        [wv16.reshape(cc, 128, H), wq16.reshape(cc, 128, H)], axis=2
    ).transpose(1, 0, 2)
    wqk = np.ascontiguousarray(wqk).astype(NP_BF16)
    wvq = np.ascontiguousarray(wvq).astype(NP_BF16)
    n_cores = x.shape[0] // b_loc
    xs = np.asarray(x, dtype=np.float32).reshape(n_cores, b_loc, t, C)
    in_maps = []
    for m in range(n_cores):
        xT = np.ascontiguousarray(xs[m].transpose(0, 2, 1)).astype(NP_BF16)
        in_maps.append({
            "xT": xT, "Wqk": wqk, "Wvq": wvq,
            "ident16": ident16, "ident32": ident32, "mask": mask,
        })
    return in_maps


def _run(x, Wk, Wq, Wv, trace=False, **spmd_kwargs):
    nc = build_nc()
    in_maps = _shard_inputs(x, Wk, Wq, Wv)
    res = run_bass_kernel_spmd(
        nc, in_maps, core_ids=list(range(N_CORES)), trace=trace, **spmd_kwargs
    )
    out = np.concatenate([res.results[m]["out"] for m in range(N_CORES)], axis=0)
    return np.ascontiguousarray(out, dtype=np.float32), res


def kernel(x, Wk, Wq, Wv):
    out, _ = _run(x, Wk, Wq, Wv)
    return out
